# revision 1
# baseline (speedup 1.0000x reference)
"""MiniMHSA Trainium2 kernel: 8 NeuronCores, shard = (batch n, head-group).

Reference computes, per batch n:
  qkv = x @ W_qkv.T + b_qkv ; split into q,k,v heads (H=16, HD=64)
  scores = (q @ k.T) / sqrt(HD), masked keys -> -1e9, softmax, @ v
  out = attn_out @ W_out.T + b_out

Core c handles n = c//2 and head-group hg = c%2 (8 heads each). Device-side
dataflow (all matmuls float32r = TF32-like, 4x faster than fp32 on the PE):

  xT [D, L] (host-transposed), W slices host-transposed/scaled:
  1) qkT  = Wqk.T @ x.T   -> q,k transposed [64*16cols, L] (+ bias per-partition)
  2) v    = x @ Wv        -> v natural [L, 512] + ones column (softmax denom)
  3) S^T  = kT.T @ qT     -> scores with k on partitions; mask = per-partition
     bias of the exp ACTIVATE (masked rows underflow to 0); no max-subtraction
     (scores are O(5), exp is safe in fp32)
  4) O^T  = v'.T @ P^T    -> [65, L] accumulated over k chunks; row 64 = sums
  5) normalize via reciprocal + rank-1 ones-outer-product broadcast
  6) y    = otn.T @ Wo + b_out (partial over this core's heads)

Host sums the two head-group partials per batch.
"""
import sys

sys.path.insert(0, '/opt/trn_rl_repo')


import numpy as np

_KERNEL_CACHE = {}


def _split_excess_waits(nc):
    """Walrus codegen reliably accepts only ONE sync wait per instruction
    (Matmult hard-fails at 2, Drain at 5). Tile's scheduler can attach more.
    Move excess waits onto preceding same-engine NOPs — semantically identical
    since engine queues execute in order."""
    from concourse import mybir

    for f in nc.m.functions:
        for blk in f.blocks:
            il = blk.instructions
            i = 0
            while i < len(il):
                inst = il[i]
                si = inst.sync_info
                waits = list(si.on_wait) if si is not None and si.on_wait else []
                if len(waits) > 1:
                    keep = waits[-1:]
                    excess = waits[:-1]
                    pos = i
                    for j, wcond in enumerate(excess):
                        nop = mybir.InstNoOp(name=f"{inst.name}-ws{j}", ins=[], outs=[])
                        nop.engine = inst.engine
                        nop.sync_info = mybir.SyncInfo(on_wait=[wcond], on_update=[])
                        il.insert(pos, nop)
                        pos += 1
                        i += 1
                    inst.sync_info = mybir.SyncInfo(
                        on_wait=keep,
                        on_update=list(si.on_update) if si.on_update else [],
                    )
                i += 1


def _build(cfg, waitsplit=True, debug_out=None):
    import concourse.bass as bass
    import concourse.tile as tile
    from concourse import mybir

    F32 = mybir.dt.float32
    F32R = mybir.dt.float32r
    AF = mybir.ActivationFunctionType
    MULT = mybir.AluOpType.mult

    L, D, HC, HD = cfg["L"], cfg["D"], cfg["HC"], cfg["HD"]
    DCH = D // 128            # contraction chunks for projections
    DV = HC * HD              # qkv width per core
    QKC = 2 * DV // 128       # qkT M-chunks total (q then k)
    QK2 = DV // 128           # global chunks holding q
    LC = L // 512             # proj L chunks
    KC = L // 128             # attention k chunks
    QH = L // 1024            # attention q hemis (1024 wide)
    PAIRS = HC // 2
    DOUT = D
    DC = DOUT // 512
    PHASES = cfg.get('PHASES', 2)
    HP = HC // PHASES         # heads per phase
    DVP = HP * HD             # qkv width per phase
    QKP = DVP // 128          # q chunks per phase
    QKCP = 2 * QKP            # local M-chunks per phase

    from concourse.tile_rust import add_dep_helper

    nc = bass.Bass()
    xT_d = nc.dram_tensor("xT", [D, L], F32, kind="ExternalInput")
    wqk_d = nc.dram_tensor("wqk", [128, DCH, 2 * DV], F32, kind="ExternalInput")
    wv_d = nc.dram_tensor("wv", [128, DCH, DV], F32, kind="ExternalInput")
    bqk_d = nc.dram_tensor("bqk", [128, QKC], F32, kind="ExternalInput")
    bv_d = nc.dram_tensor("bv", [1, DV], F32, kind="ExternalInput")
    mb_d = nc.dram_tensor("mb", [128, KC], F32, kind="ExternalInput")
    wo_d = nc.dram_tensor("wo", [64, HC, DOUT], F32, kind="ExternalInput")
    bo_d = nc.dram_tensor("bo", [1, DOUT], F32, kind="ExternalInput")
    y_d = nc.dram_tensor("y", [L, DOUT], F32, kind="ExternalOutput")
    otn_dram = nc.dram_tensor("otn_bounce", [HC, 64, L], F32)  # internal
    otn_d = None
    if debug_out == "otn":
        otn_d = nc.dram_tensor("otn_o", [HC, 64, L], F32, kind="ExternalOutput")

    with tile.TileContext(nc) as tc, \
         nc.allow_low_precision(reason="float32r matmuls intended"):
        with tc.tile_pool(name="const", bufs=1) as const, \
             tc.tile_pool(name="workP", bufs=4) as workP, \
             tc.tile_pool(name="workS", bufs=2) as workS:

            # ---- constants ----
            bqk_t = const.tile([128, QKC], F32)
            nc.sync.dma_start(out=bqk_t, in_=bqk_d[:, :])
            mb_t = const.tile([128, KC], F32)
            nc.sync.dma_start(out=mb_t, in_=mb_d[:, :])
            bv_r = const.tile([1, DV], F32R)
            nc.gpsimd.dma_start(out=bv_r, in_=bv_d[:, :])
            bo_r = const.tile([1, DOUT], F32R)
            nc.gpsimd.dma_start(out=bo_r, in_=bo_d[:, :])
            ones_f = const.tile([128, 1], F32)
            nc.vector.memset(ones_f, 1.0)
            ones_r = const.tile([1, 128], F32R)
            nc.vector.tensor_copy(out=ones_r, in_=ones_f[0:1, 0:1].broadcast_to([1, 128]))

            bounce_insts = []
            for ph in range(PHASES):
                with tc.tile_pool(name=f"big{ph}", bufs=1) as big:
                    qkT_r = big.tile([128, QKCP, L], F32R, tag="qkT")
                    vp_r = big.tile([128, KC, HP, HD + 1], F32R, tag="vp")
                    nc.vector.tensor_copy(
                        out=vp_r[:, :, :, HD:HD + 1],
                        in_=ones_f.unsqueeze(1).unsqueeze(1).broadcast_to([128, KC, HP, 1]),
                    )

                    # ---- projections (this phase's heads) ----
                    with tc.tile_pool(name=f"w{ph}", bufs=1) as wpool, \
                         tc.tile_pool(name=f"xt{ph}", bufs=2) as xtpool, \
                         tc.tile_pool(name=f"psA{ph}", bufs=2, space="PSUM") as psA:
                        wqk_r = wpool.tile([128, DCH, 2 * DVP], F32R, tag="wqk")
                        nc.gpsimd.dma_start(
                            out=wqk_r[:, :, 0:DVP],
                            in_=wqk_d[:, :, ph * DVP:(ph + 1) * DVP])
                        nc.gpsimd.dma_start(
                            out=wqk_r[:, :, DVP:2 * DVP],
                            in_=wqk_d[:, :, DV + ph * DVP:DV + (ph + 1) * DVP])
                        wv_r = wpool.tile([128, DCH, DVP], F32R, tag="wv")
                        nc.gpsimd.dma_start(
                            out=wv_r, in_=wv_d[:, :, ph * DVP:(ph + 1) * DVP])

                        for lc in range(LC):
                            xt_r = xtpool.tile([128, DCH, 512], F32R)
                            nc.gpsimd.dma_start(
                                out=xt_r,
                                in_=xT_d.rearrange("(c p) l -> p c l", p=128)[:, :, lc * 512:(lc + 1) * 512],
                            )
                            for mc in range(QKCP):
                                gcol = (ph * QKP + mc) if mc < QKP \
                                    else (QK2 + ph * QKP + (mc - QKP))
                                qk_ps = psA.tile([128, 512], F32, tag="qk")
                                for k in range(DCH):
                                    nc.tensor.matmul(
                                        qk_ps[:, :],
                                        wqk_r[:, k, mc * 128:(mc + 1) * 128],
                                        xt_r[:, k, :],
                                        start=(k == 0), stop=(k == DCH - 1),
                                    )
                                nc.vector.tensor_scalar_add(
                                    out=qkT_r[:, mc, lc * 512:(lc + 1) * 512],
                                    in0=qk_ps, scalar1=bqk_t[:, gcol:gcol + 1],
                                )
                            for sub in range(4):
                                v_ps = psA.tile([128, DVP], F32, tag="v")
                                for k in range(DCH):
                                    nc.tensor.matmul(
                                        v_ps[:, :],
                                        xt_r[:, k, sub * 128:(sub + 1) * 128],
                                        wv_r[:, k, :],
                                        start=(k == 0), stop=False,
                                    )
                                nc.tensor.matmul(
                                    v_ps[:, :], ones_r[0:1, :],
                                    bv_r[0:1, ph * DVP:(ph + 1) * DVP],
                                    start=False, stop=True,
                                )
                                kcg = lc * 4 + sub
                                nc.vector.tensor_copy(
                                    out=vp_r[:, kcg, :, 0:HD],
                                    in_=v_ps.rearrange("p (h d) -> p h d", h=HP),
                                )

                    # ---- attention (this phase's heads) ----
                    with tc.tile_pool(name=f"psB{ph}", bufs=2, space="PSUM") as psB, \
                         tc.tile_pool(name=f"psC{ph}", bufs=2, space="PSUM") as psC:
                        for hl in range(HP):
                            hg = ph * HP + hl
                            base = (hl % 2) * 64
                            kchunk = QKP + hl // 2
                            qchunk = hl // 2
                            for qh in range(QH):
                                q0 = qh * 1024
                                ot_ps = psC.tile([HD + 1, 1024], F32, tag="ot")
                                for kc in range(KC):
                                    st_ps = psB.tile([128, 1024], F32, tag="st")
                                    for s in range(2):
                                        nc.tensor.matmul(
                                            st_ps[:, s * 512:(s + 1) * 512],
                                            qkT_r[base:base + 64, kchunk, kc * 128:(kc + 1) * 128],
                                            qkT_r[base:base + 64, qchunk, q0 + s * 512:q0 + (s + 1) * 512],
                                            start=True, stop=True,
                                        )
                                    pT = workP.tile([128, 1024], F32R, tag="pT")
                                    nc.scalar.activation(
                                        out=pT, in_=st_ps, func=AF.Exp,
                                        bias=mb_t[:, kc:kc + 1], scale=1.0,
                                    )
                                    for s in range(2):
                                        nc.tensor.matmul(
                                            ot_ps[:, s * 512:(s + 1) * 512],
                                            vp_r[:, kc, hl, :],
                                            pT[:, s * 512:(s + 1) * 512],
                                            start=(kc == 0), stop=(kc == KC - 1),
                                        )
                                recip_r = workS.tile([1, 1024], F32R, tag="recip")
                                nc.vector.reciprocal(out=recip_r, in_=ot_ps[HD:HD + 1, :])
                                bc_ps = psB.tile([64, 1024], F32, tag="st")
                                for s in range(2):
                                    nc.tensor.matmul(
                                        bc_ps[:, s * 512:(s + 1) * 512],
                                        ones_r[0:1, 0:64],
                                        recip_r[0:1, s * 512:(s + 1) * 512],
                                        start=True, stop=True,
                                    )
                                bc_sb = workS.tile([64, 1024], F32, tag="bc")
                                nc.vector.tensor_copy(out=bc_sb, in_=bc_ps)
                                otn_sb = workS.tile([64, 1024], F32, tag="otn")
                                nc.vector.tensor_tensor(
                                    out=otn_sb, in0=ot_ps[0:HD, :], in1=bc_sb, op=MULT,
                                )
                                _bi = nc.sync.dma_start(
                                    out=otn_dram[hg, :, q0:q0 + 1024], in_=otn_sb)
                                bounce_insts.append(_bi.ins)

            if debug_out == "otn":
                nc.gpsimd.dma_start(out=otn_d[:, :, :], in_=otn_dram[:, :, :])
            if debug_out == "stop_after_attn":
                pass
            # ---- output projection ----
            if debug_out == "stop_after_attn":
                skip_outproj = True
            else:
                skip_outproj = False
            with tc.tile_pool(name="wo", bufs=1) as wopool, \
                 tc.tile_pool(name="psD", bufs=4, space="PSUM") as psD:
                if skip_outproj:
                    zz = wopool.tile([128, DOUT], F32)
                    nc.vector.memset(zz, 0.0)
                    nc.sync.dma_start(out=y_d[0:128, :], in_=zz)
                else:
                    wo_r = wopool.tile([64, HC, DOUT], F32R)
                    nc.gpsimd.dma_start(out=wo_r, in_=wo_d[:, :, :])
                    otn_pr = wopool.tile([64, HC, L], F32R)
                    _rb = nc.gpsimd.dma_start(
                        out=otn_pr, in_=otn_dram.rearrange("h p l -> p h l"))
                    for _bi in bounce_insts:
                        add_dep_helper(_rb.ins, _bi, sync=True,
                                       reason="otn readback waits on bounces")
                for qt in range(0 if skip_outproj else L // 128):
                    y_sb = workS.tile([128, DOUT], F32, tag="y")
                    for dc in range(DC):
                        y_ps = psD.tile([128, 512], F32, tag="y")
                        for h in range(HC):
                            nc.tensor.matmul(
                                y_ps[:, :],
                                otn_pr[:, h, qt * 128:(qt + 1) * 128],
                                wo_r[:, h, dc * 512:(dc + 1) * 512],
                                start=(h == 0), stop=False,
                            )
                        nc.tensor.matmul(
                            y_ps[:, :], ones_r[0:1, :], bo_r[0:1, dc * 512:(dc + 1) * 512],
                            start=False, stop=True,
                        )
                        nc.vector.tensor_copy(
                            out=y_sb[:, dc * 512:(dc + 1) * 512], in_=y_ps,
                        )
                    nc.sync.dma_start(out=y_d[qt * 128:(qt + 1) * 128, :], in_=y_sb)

    # split multi-waits (walrus allows 1 sync wait per instruction reliably)
    if waitsplit:
        _split_excess_waits(nc)
    return nc


def _prep_inputs(x, mask, W_qkv, b_qkv, W_out, b_out, cfg):
    """Build the 8 per-core input maps (host-side shuffles, float32)."""
    L, D, HC, HD = cfg["L"], cfg["D"], cfg["HC"], cfg["HD"]
    DV = HC * HD
    N = x.shape[0]
    scale = 1.0 / np.sqrt(HD)
    Wt = np.ascontiguousarray(W_qkv.T).astype(np.float32)    # [D, 3D]
    WoT = np.ascontiguousarray(W_out.T).astype(np.float32)   # [D, D]
    DCH = D // 128
    QKC = 2 * DV // 128
    KC = L // 128
    PAIRS = HC // 2

    per_hg = []
    for hg in range(2):
        qs, ks, vs = hg * DV, D + hg * DV, 2 * D + hg * DV
        wqk = np.concatenate(
            [Wt[:, qs:qs + DV] * scale, Wt[:, ks:ks + DV]], axis=1
        )  # [D, 2DV]
        wqk = wqk.reshape(DCH, 128, 2 * DV)  # [c, p, cols]
        wqk = np.ascontiguousarray(wqk.transpose(1, 0, 2))  # [128, c, cols]
        wv = Wt[:, vs:vs + DV].reshape(DCH, 128, DV)
        wv = np.ascontiguousarray(wv.transpose(1, 0, 2))
        bqk = np.concatenate(
            [b_qkv[qs:qs + DV] * scale, b_qkv[ks:ks + DV]]
        ).reshape(QKC, 128)
        bqk = np.ascontiguousarray(bqk.T)  # [128, QKC]
        bv = np.ascontiguousarray(b_qkv[vs:vs + DV][None, :])
        # wo: [HD, HC, D] — per-head rows, partition base 0 only (mixing
        # stationary partition bases within one PSUM accumulation group
        # crashes the exec unit)
        wo_heads = WoT[hg * DV:(hg + 1) * DV, :].reshape(HC, HD, D)
        wo = np.ascontiguousarray(wo_heads.transpose(1, 0, 2))
        per_hg.append(dict(wqk=wqk, wv=wv, bqk=bqk, bv=bv, wo=wo))

    # b_out only on hg=0 cores; partials are summed on host (avoid 2x bias)
    bo_full = np.ascontiguousarray(b_out[None, :]).astype(np.float32)
    bo_zero = np.zeros_like(bo_full)
    xTs, mbs = [], []
    for n in range(N):
        xTs.append(np.ascontiguousarray(x[n].T).astype(np.float32))
        mb = np.where(mask[n], np.float32(-1e9), np.float32(0.0))
        mbs.append(np.ascontiguousarray(mb.reshape(KC, 128).T))

    in_maps = []
    for c in range(2 * N):
        n, hg = c // 2, c % 2
        d = dict(per_hg[hg])
        d.update(xT=xTs[n], mb=mbs[n], bo=(bo_full if hg == 0 else bo_zero))
        in_maps.append(d)
    return in_maps


def kernel(x, mask, W_qkv, b_qkv, W_out, b_out):
    from concourse.bass_utils import run_bass_kernel_spmd

    x = np.asarray(x, dtype=np.float32)
    mask = np.asarray(mask)
    N, L, D = x.shape
    H = 16
    HD = D // H
    cfg = {"L": L, "D": D, "HC": H // 2, "HD": HD}

    key = (L, D, H)
    if key not in _KERNEL_CACHE:
        _KERNEL_CACHE[key] = _build(cfg)
    nc = _KERNEL_CACHE[key]

    in_maps = _prep_inputs(
        x, mask,
        np.asarray(W_qkv, np.float32), np.asarray(b_qkv, np.float32),
        np.asarray(W_out, np.float32), np.asarray(b_out, np.float32), cfg,
    )
    res = run_bass_kernel_spmd(nc, in_maps, list(range(2 * N)))
    out = np.empty((N, L, D), np.float32)
    for n in range(N):
        out[n] = res.results[2 * n]["y"] + res.results[2 * n + 1]["y"]
    return out



# revision 6
# speedup vs baseline: 1.7377x; 1.7377x over previous
"""MiniMHSA Trainium2 kernel: 8 NeuronCores, shard = (batch n, head-group).

Reference computes, per batch n:
  qkv = x @ W_qkv.T + b_qkv ; split into q,k,v heads (H=16, HD=64)
  scores = (q @ k.T) / sqrt(HD), masked keys -> -1e9, softmax, @ v
  out = attn_out @ W_out.T + b_out

Core c handles n = c//2 and head-group hg = c%2 (8 heads each).

Key optimizations over the naive dataflow:
  * Mask compaction: masked keys contribute exactly 0 to softmax, so k/v are
    only computed for the ~50% unmasked keys (host gathers x columns, pads to
    a multiple of 128). Scores/PV/k-proj/v-proj all shrink proportionally.
  * Head-pair packing: two heads share the 128 partitions everywhere
    (partition p = (h%2)*64 + dim). Out-projection contracts 4 chunks of 128
    instead of 8 of 64.
  * v-bias + out-bias folded on host: softmax weights sum to 1, so the v bias
    contributes bv @ W_out.T — a constant folded into an effective out bias.
    No bias matmuls on device for v; out bias added via a one-time PE
    broadcast + free DVE add on the PSUM->SBUF copy.
  * bf16 for qT/kT/vp/pT (attention operands): halves SBUF, same PE cost.
  * No max-subtraction softmax: exp(s + mask_bias) directly (scores are O(5));
    denominator via an appended ones-column in v.
  * Software-pipelined attention: S(kc+1) issued before PV(kc) so the PE can
    run ahead of the exp activations.
"""
import sys

sys.path.insert(0, '/opt/trn_rl_repo')


import numpy as np

_KERNEL_CACHE = {}


def _split_excess_waits(nc):
    """Walrus codegen reliably accepts only ONE sync wait per instruction
    (Matmult hard-fails at 2, Drain at 5). Tile's scheduler can attach more.
    Move excess waits onto preceding same-engine NOPs — semantically identical
    since engine queues execute in order."""
    from concourse import mybir

    for f in nc.m.functions:
        for blk in f.blocks:
            il = blk.instructions
            i = 0
            while i < len(il):
                inst = il[i]
                si = inst.sync_info
                waits = list(si.on_wait) if si is not None and si.on_wait else []
                if len(waits) > 1:
                    keep = waits[-1:]
                    excess = waits[:-1]
                    pos = i
                    for j, wcond in enumerate(excess):
                        nop = mybir.InstNoOp(name=f"{inst.name}-ws{j}", ins=[], outs=[])
                        nop.engine = inst.engine
                        nop.sync_info = mybir.SyncInfo(on_wait=[wcond], on_update=[])
                        il.insert(pos, nop)
                        pos += 1
                        i += 1
                    inst.sync_info = mybir.SyncInfo(
                        on_wait=keep,
                        on_update=list(si.on_update) if si.on_update else [],
                    )
                i += 1


def _col_chunks(total):
    """Split a multiple-of-128 column count into chunks of 256..512 (each a
    multiple of 128) so fp32r matmuls stay >= 256 moving columns."""
    out, rem = [], total
    while rem > 512:
        step = 512 if rem - 512 >= 256 else 384
        out.append(step)
        rem -= step
    out.append(rem)
    return out


def _build(cfg, waitsplit=True):
    import concourse.bass as bass
    import concourse.tile as tile
    from concourse import mybir

    F32 = mybir.dt.float32
    F32R = mybir.dt.float32r
    BF16 = mybir.dt.bfloat16
    AF = mybir.ActivationFunctionType
    MULT = mybir.AluOpType.mult
    ADD = mybir.AluOpType.add

    L, D, KPC = cfg["L"], cfg["D"], cfg["KPC"]
    HC, HD = cfg["HC"], cfg["HD"]
    DCH = D // 128            # contraction chunks for projections
    PAIRS = HC // 2
    KPAD = KPC * 128
    QH = L // 1024            # attention q hemis (1024 wide)
    DOUT = D
    DC = DOUT // 512

    nc = bass.Bass()
    xT_d = nc.dram_tensor("xT", [D, L], BF16, kind="ExternalInput")
    xTk_d = nc.dram_tensor("xTk", [D, KPAD], BF16, kind="ExternalInput")
    wqk_d = nc.dram_tensor("wqk", [128, DCH, 1024], BF16, kind="ExternalInput")
    wv_d = nc.dram_tensor("wv", [128, DCH, 512], BF16, kind="ExternalInput")
    bqk_d = nc.dram_tensor("bqk", [128, 2 * PAIRS], F32, kind="ExternalInput")
    mb_d = nc.dram_tensor("mb", [128, KPC], F32, kind="ExternalInput")
    wo_d = nc.dram_tensor("wo", [128, PAIRS, DOUT], F32, kind="ExternalInput")
    bo_d = nc.dram_tensor("bo", [1, DOUT], F32, kind="ExternalInput")
    y_d = nc.dram_tensor("y", [L, DOUT], F32, kind="ExternalOutput")

    with tile.TileContext(nc) as tc, \
         nc.allow_low_precision(reason="float32r/bf16 matmuls intended"):
        with tc.tile_pool(name="const", bufs=1) as const, \
             tc.tile_pool(name="big", bufs=1) as big, \
             tc.tile_pool(name="workP", bufs=4) as workP, \
             tc.tile_pool(name="workS", bufs=2) as workS:

            # ---- constants ----
            bqk_t = const.tile([128, 2 * PAIRS], F32)
            nc.gpsimd.dma_start(out=bqk_t, in_=bqk_d[:, :])
            mb_t = const.tile([128, KPC], F32)
            nc.gpsimd.dma_start(out=mb_t, in_=mb_d[:, :])
            bo_t = const.tile([1, DOUT], F32R)
            nc.gpsimd.dma_start(out=bo_t, in_=bo_d[:, :])
            ones_f = const.tile([128, 1], F32)
            nc.vector.memset(ones_f, 1.0)
            ones_r = const.tile([1, 128], F32R)
            nc.vector.tensor_copy(out=ones_r, in_=ones_f[0:1, 0:1].broadcast_to([1, 128]))
            bo_bc = const.tile([128, DOUT], F32)

            qT = big.tile([128, PAIRS, L], BF16, tag="qT")
            kT = big.tile([128, PAIRS, KPAD], BF16, tag="kT")
            vp = big.tile([128, KPC, HC, HD + 1], BF16, tag="vp")
            otn = big.tile([128, PAIRS, L], F32R, tag="otn")
            nc.vector.tensor_copy(
                out=vp[:, :, :, HD:HD + 1],
                in_=ones_f.unsqueeze(1).unsqueeze(1).broadcast_to([128, KPC, HC, 1]),
            )

            # ---- projections ----
            with tc.tile_pool(name="w", bufs=1) as wpool, \
                 tc.tile_pool(name="xt", bufs=2) as xtp, \
                 tc.tile_pool(name="xtk", bufs=2) as xtkp, \
                 tc.tile_pool(name="psA", bufs=2, space="PSUM") as psA, \
                 tc.tile_pool(name="psV", bufs=2, space="PSUM") as psV:
                wqk_r = wpool.tile([128, DCH, 1024], BF16, tag="wqk")
                nc.gpsimd.dma_start(out=wqk_r, in_=wqk_d[:, :, :])
                wv_r = wpool.tile([128, DCH, 512], BF16, tag="wv")
                nc.gpsimd.dma_start(out=wv_r, in_=wv_d[:, :, :])

                # out-bias broadcast [1, D] -> [128, D] (one-time)
                for s in range(DC):
                    bo_ps = psA.tile([128, 512], F32, tag="qk")
                    nc.tensor.matmul(bo_ps, ones_r[0:1, :], bo_t[0:1, s * 512:(s + 1) * 512],
                                     start=True, stop=True)
                    nc.vector.tensor_copy(out=bo_bc[:, s * 512:(s + 1) * 512], in_=bo_ps)

                # k/v projections from compacted keys
                c0 = 0
                for w in _col_chunks(KPAD):
                    xtk_r = xtkp.tile([128, DCH, w], BF16, tag="xtk")
                    nc.sync.dma_start(
                        out=xtk_r,
                        in_=xTk_d.rearrange("(c p) l -> p c l", p=128)[:, :, c0:c0 + w])
                    for mc in range(PAIRS):
                        ps = psA.tile([128, w], F32, tag="qk")
                        for k in range(DCH):
                            nc.tensor.matmul(
                                ps, wqk_r[:, k, 512 + mc * 128:512 + (mc + 1) * 128],
                                xtk_r[:, k, :], start=(k == 0), stop=(k == DCH - 1))
                        nc.vector.tensor_scalar_add(
                            out=kT[:, mc, c0:c0 + w], in0=ps,
                            scalar1=bqk_t[:, PAIRS + mc:PAIRS + mc + 1])
                    for sub in range(w // 128):
                        kcg = c0 // 128 + sub
                        psv = psV.tile([128, 512], F32, tag="v")
                        for k in range(DCH):
                            nc.tensor.matmul(
                                psv, xtk_r[:, k, sub * 128:(sub + 1) * 128],
                                wv_r[:, k, :], start=(k == 0), stop=(k == DCH - 1))
                        nc.vector.tensor_copy(
                            out=vp[:, kcg, :, 0:HD],
                            in_=psv.rearrange("p (h d) -> p h d", h=HC))
                    c0 += w

                # q projection (all L positions)
                for lc in range(L // 512):
                    xt_r = xtp.tile([128, DCH, 512], BF16, tag="xt")
                    nc.sync.dma_start(
                        out=xt_r,
                        in_=xT_d.rearrange("(c p) l -> p c l", p=128)[:, :, lc * 512:(lc + 1) * 512])
                    for mc in range(PAIRS):
                        ps = psA.tile([128, 512], F32, tag="qk")
                        for k in range(DCH):
                            nc.tensor.matmul(
                                ps, wqk_r[:, k, mc * 128:(mc + 1) * 128],
                                xt_r[:, k, :], start=(k == 0), stop=(k == DCH - 1))
                        nc.vector.tensor_scalar_add(
                            out=qT[:, mc, lc * 512:(lc + 1) * 512], in0=ps,
                            scalar1=bqk_t[:, mc:mc + 1])

            # ---- attention ----
            with tc.tile_pool(name="psB", bufs=2, space="PSUM") as psB, \
                 tc.tile_pool(name="psC", bufs=2, space="PSUM") as psC:
                for h in range(HC):
                    i, j = h // 2, h % 2
                    base = j * 64
                    for qh in range(QH):
                        q0 = qh * 1024
                        ot = psC.tile([HD + 1, 1024], F32, tag="ot")

                        def emit_pv(kc, pT):
                            for s in range(2):
                                nc.tensor.matmul(
                                    ot[:, s * 512:(s + 1) * 512],
                                    vp[:, kc, h, :], pT[:, s * 512:(s + 1) * 512],
                                    start=(kc == 0), stop=(kc == KPC - 1))

                        prev = None
                        for kc in range(KPC):
                            st = psB.tile([128, 1024], F32, tag="st")
                            for s in range(2):
                                nc.tensor.matmul(
                                    st[:, s * 512:(s + 1) * 512],
                                    kT[base:base + 64, i, kc * 128:(kc + 1) * 128],
                                    qT[base:base + 64, i, q0 + s * 512:q0 + (s + 1) * 512],
                                    start=True, stop=True)
                            if prev is not None:
                                emit_pv(kc - 1, prev)
                            pT = workP.tile([128, 1024], BF16, tag="pT")
                            nc.scalar.activation(
                                out=pT, in_=st, func=AF.Exp,
                                bias=mb_t[:, kc:kc + 1], scale=1.0)
                            prev = pT
                        emit_pv(KPC - 1, prev)

                        recip = workS.tile([1, 1024], F32R, tag="recip")
                        nc.vector.reciprocal(out=recip, in_=ot[HD:HD + 1, :])
                        bc_ps = psB.tile([64, 1024], F32, tag="st")
                        for s in range(2):
                            nc.tensor.matmul(
                                bc_ps[:, s * 512:(s + 1) * 512], ones_r[0:1, 0:64],
                                recip[0:1, s * 512:(s + 1) * 512], start=True, stop=True)
                        bc_sb = workS.tile([64, 1024], F32, tag="bc")
                        nc.vector.tensor_copy(out=bc_sb, in_=bc_ps)
                        nc.vector.tensor_tensor(
                            out=otn[base:base + 64, i, q0:q0 + 1024],
                            in0=ot[0:HD, :], in1=bc_sb, op=MULT)

            # ---- output projection ----
            with tc.tile_pool(name="wo", bufs=1) as wopool, \
                 tc.tile_pool(name="psD", bufs=4, space="PSUM") as psD:
                wo_r = wopool.tile([128, PAIRS, DOUT], F32R)
                nc.gpsimd.dma_start(out=wo_r, in_=wo_d[:, :, :])
                for qt in range(L // 128):
                    y_sb = workS.tile([128, DOUT], F32, tag="y")
                    for dc in range(DC):
                        y_ps = psD.tile([128, 512], F32, tag="y")
                        for i2 in range(PAIRS):
                            nc.tensor.matmul(
                                y_ps, otn[:, i2, qt * 128:(qt + 1) * 128],
                                wo_r[:, i2, dc * 512:(dc + 1) * 512],
                                start=(i2 == 0), stop=(i2 == PAIRS - 1))
                        nc.vector.tensor_tensor(
                            out=y_sb[:, dc * 512:(dc + 1) * 512],
                            in0=y_ps, in1=bo_bc[:, dc * 512:(dc + 1) * 512], op=ADD)
                    nc.sync.dma_start(out=y_d[qt * 128:(qt + 1) * 128, :], in_=y_sb)

    # split multi-waits (walrus allows 1 sync wait per instruction reliably)
    if waitsplit:
        _split_excess_waits(nc)
    return nc


def _prep_inputs(x, mask, W_qkv, b_qkv, W_out, b_out, cfg):
    """Build the 8 per-core input maps (host-side shuffles, float32)."""
    import ml_dtypes
    BF = ml_dtypes.bfloat16

    L, D, KPC = cfg["L"], cfg["D"], cfg["KPC"]
    HC, HD = cfg["HC"], cfg["HD"]
    DV = HC * HD              # 512 qkv dims per head-group
    KPAD = KPC * 128
    N = x.shape[0]
    scale = np.float32(1.0 / np.sqrt(HD))
    Wt = np.ascontiguousarray(W_qkv.T).astype(np.float32)    # [D, 3D]
    WoT = np.ascontiguousarray(W_out.T).astype(np.float32)   # [D, D]
    DCH = D // 128
    PAIRS = HC // 2

    # head-pair permutation: chunk mc, col c -> head 2mc + c//64, dim c%64
    idx = np.empty((PAIRS, 128), np.int64)
    for mc in range(PAIRS):
        c = np.arange(128)
        idx[mc] = (2 * mc + c // 64) * 64 + (c % 64)
    idxf = idx.reshape(-1)

    per_hg = []
    for hg in range(2):
        qs, ks, vs = hg * DV, D + hg * DV, 2 * D + hg * DV
        wq = Wt[:, qs:qs + DV][:, idxf] * scale
        wk = Wt[:, ks:ks + DV][:, idxf]
        wqk = np.concatenate([wq, wk], axis=1)                    # [D, 1024]
        wqk = np.ascontiguousarray(
            wqk.reshape(DCH, 128, 2 * DV).transpose(1, 0, 2)).astype(BF)
        wv = Wt[:, vs:vs + DV].reshape(DCH, 128, DV)
        wv = np.ascontiguousarray(wv.transpose(1, 0, 2)).astype(BF)
        bq = b_qkv[qs:qs + DV][idxf] * scale
        bk = b_qkv[ks:ks + DV][idxf]
        bqk = np.stack(
            [bq[mc * 128:(mc + 1) * 128] for mc in range(PAIRS)]
            + [bk[mc * 128:(mc + 1) * 128] for mc in range(PAIRS)], axis=1)
        bqk = np.ascontiguousarray(bqk).astype(np.float32)        # [128, 2*PAIRS]
        WoT_blk = WoT[hg * DV:(hg + 1) * DV, :]                   # [512, D]
        wo = np.ascontiguousarray(
            np.stack([WoT_blk[idx[i], :] for i in range(PAIRS)], axis=1))  # [128,4,D]
        # v-bias folds through softmax (weights sum to 1): bv @ WoT_blk
        bv = b_qkv[vs:vs + DV].astype(np.float32)
        bo_eff = bv @ WoT_blk
        if hg == 0:
            bo_eff = bo_eff + b_out.astype(np.float32)
        bo_eff = np.ascontiguousarray(bo_eff[None, :]).astype(np.float32)
        per_hg.append(dict(wqk=wqk, wv=wv, bqk=bqk, wo=wo, bo=bo_eff))

    xTs, xTks, mbs = [], [], []
    for n in range(N):
        xTs.append(np.ascontiguousarray(x[n].T).astype(BF))
        kept = np.nonzero(~mask[n])[0]
        xk = np.zeros((KPAD, D), np.float32)
        xk[:len(kept)] = x[n][kept]
        xTks.append(np.ascontiguousarray(xk.T).astype(BF))
        mb = np.full(KPAD, -1e9, np.float32)
        mb[:len(kept)] = 0.0
        mbs.append(np.ascontiguousarray(mb.reshape(KPC, 128).T))

    in_maps = []
    for c in range(2 * N):
        n, hg = c // 2, c % 2
        d = dict(per_hg[hg])
        d.update(xT=xTs[n], xTk=xTks[n], mb=mbs[n])
        in_maps.append(d)
    return in_maps


def kernel(x, mask, W_qkv, b_qkv, W_out, b_out):
    from concourse.bass_utils import run_bass_kernel_spmd

    x = np.asarray(x, dtype=np.float32)
    mask = np.asarray(mask).astype(bool)
    N, L, D = x.shape
    H = 16
    HD = D // H
    kept_max = int((~mask).sum(axis=1).max())
    KPC = max(2, -(-kept_max // 128))
    cfg = {"L": L, "D": D, "HC": H // 2, "HD": HD, "KPC": KPC}

    key = (L, D, H, KPC)
    if key not in _KERNEL_CACHE:
        _KERNEL_CACHE[key] = _build(cfg)
    nc = _KERNEL_CACHE[key]

    in_maps = _prep_inputs(
        x, mask,
        np.asarray(W_qkv, np.float32), np.asarray(b_qkv, np.float32),
        np.asarray(W_out, np.float32), np.asarray(b_out, np.float32), cfg,
    )
    res = run_bass_kernel_spmd(nc, in_maps, list(range(2 * N)))
    out = np.empty((N, L, D), np.float32)
    for n in range(N):
        out[n] = res.results[2 * n]["y"] + res.results[2 * n + 1]["y"]
    return out


# revision 7
# speedup vs baseline: 2.0654x; 1.1886x over previous
"""MiniMHSA Trainium2 kernel: 8 NeuronCores, shard = (batch n, head-group).

Reference computes, per batch n:
  qkv = x @ W_qkv.T + b_qkv ; split into q,k,v heads (H=16, HD=64)
  scores = (q @ k.T) / sqrt(HD), masked keys -> -1e9, softmax, @ v
  out = attn_out @ W_out.T + b_out

Core c handles n = c//2 and head-group hg = c%2 (8 heads each).

Key optimizations over the naive dataflow:
  * Mask compaction: masked keys contribute exactly 0 to softmax, so k/v are
    only computed for the ~50% unmasked keys (host gathers x columns, pads to
    a multiple of 128). Scores/PV/k-proj/v-proj all shrink proportionally.
  * Head-pair packing: two heads share the 128 partitions everywhere
    (partition p = (h%2)*64 + dim). Out-projection contracts 4 chunks of 128
    instead of 8 of 64.
  * v-bias + out-bias folded on host: softmax weights sum to 1, so the v bias
    contributes bv @ W_out.T — a constant folded into an effective out bias.
  * bf16 x/weights/attention operands (PE cost identical, halves SBUF/DMA);
    f32r out-projection.
  * No max-subtraction softmax: exp(s + mask_bias) directly (scores are O(5));
    denominator via an appended ones-column in v.
  * Pipelined attention: S(kc+1) ahead of PV(kc); normalization of iteration
    i-1 emitted inside iteration i; leftover projection / out-projection
    groups interleaved as PE filler during exp-paced attention.
"""
import sys

sys.path.insert(0, '/opt/trn_rl_repo')


import numpy as np

_KERNEL_CACHE = {}


def _split_excess_waits(nc):
    """Walrus codegen reliably accepts only ONE sync wait per instruction
    (Matmult hard-fails at 2, Drain at 5). Tile's scheduler can attach more.
    Move excess waits onto preceding same-engine NOPs — semantically identical
    since engine queues execute in order."""
    from concourse import mybir

    for f in nc.m.functions:
        for blk in f.blocks:
            il = blk.instructions
            i = 0
            while i < len(il):
                inst = il[i]
                si = inst.sync_info
                waits = list(si.on_wait) if si is not None and si.on_wait else []
                if len(waits) > 1:
                    keep = waits[-1:]
                    excess = waits[:-1]
                    pos = i
                    for j, wcond in enumerate(excess):
                        nop = mybir.InstNoOp(name=f"{inst.name}-ws{j}", ins=[], outs=[])
                        nop.engine = inst.engine
                        nop.sync_info = mybir.SyncInfo(on_wait=[wcond], on_update=[])
                        il.insert(pos, nop)
                        pos += 1
                        i += 1
                    inst.sync_info = mybir.SyncInfo(
                        on_wait=keep,
                        on_update=list(si.on_update) if si.on_update else [],
                    )
                i += 1


def _col_chunks(total):
    """Split a multiple-of-128 column count into chunks of 256..512 (each a
    multiple of 128) so fp32r matmuls stay >= 256 moving columns."""
    out, rem = [], total
    while rem > 512:
        step = 512 if rem - 512 >= 256 else 384
        out.append(step)
        rem -= step
    out.append(rem)
    return out


def _build(cfg, waitsplit=True):
    import concourse.bass as bass
    import concourse.tile as tile
    from concourse import mybir

    F32 = mybir.dt.float32
    F32R = mybir.dt.float32r
    BF16 = mybir.dt.bfloat16
    AF = mybir.ActivationFunctionType
    MULT = mybir.AluOpType.mult
    ADD = mybir.AluOpType.add

    L, D, KPC = cfg["L"], cfg["D"], cfg["KPC"]
    HC, HD = cfg["HC"], cfg["HD"]
    DCH = D // 128            # contraction chunks for projections
    PAIRS = HC // 2
    KPAD = KPC * 128
    QH = L // 1024            # attention q hemis (1024 wide)
    DOUT = D
    DC = DOUT // 512
    KCH = _col_chunks(KPAD)
    KST = [0]
    for w in KCH:
        KST.append(KST[-1] + w)

    nc = bass.Bass()
    xT_d = nc.dram_tensor("xT", [D, L], BF16, kind="ExternalInput")
    xTk_d = nc.dram_tensor("xTk", [D, KPAD], BF16, kind="ExternalInput")
    wqk_d = nc.dram_tensor("wqk", [128, DCH, 1024], BF16, kind="ExternalInput")
    wv_d = nc.dram_tensor("wv", [128, DCH, 512], BF16, kind="ExternalInput")
    bqk_d = nc.dram_tensor("bqk", [128, 2 * PAIRS], F32, kind="ExternalInput")
    mb_d = nc.dram_tensor("mb", [128, KPC], F32, kind="ExternalInput")
    wo_d = nc.dram_tensor("wo", [128, PAIRS, DOUT], F32, kind="ExternalInput")
    bo_d = nc.dram_tensor("bo", [1, DOUT], F32, kind="ExternalInput")
    y_d = nc.dram_tensor("y", [L, DOUT], F32, kind="ExternalOutput")

    with tile.TileContext(nc) as tc, \
         nc.allow_low_precision(reason="float32r/bf16 matmuls intended"):
        with tc.tile_pool(name="const", bufs=1) as const, \
             tc.tile_pool(name="big", bufs=1) as big, \
             tc.tile_pool(name="xp", bufs=1) as xp, \
             tc.tile_pool(name="workP", bufs=4) as workP, \
             tc.tile_pool(name="workS", bufs=2) as workS, \
             tc.tile_pool(name="psB", bufs=2, space="PSUM") as psB, \
             tc.tile_pool(name="psC", bufs=1, space="PSUM") as psC, \
             tc.tile_pool(name="psF", bufs=2, space="PSUM") as psF:

            # ---- constants / weights (pool DMA queue; wqk first: PE waits it) ----
            wqk_r = const.tile([128, DCH, 1024], BF16, tag="wqk")
            nc.gpsimd.dma_start(out=wqk_r, in_=wqk_d[:, :, :])
            bqk_t = const.tile([128, 2 * PAIRS], F32)
            nc.gpsimd.dma_start(out=bqk_t, in_=bqk_d[:, :])
            mb_t = const.tile([128, KPC], F32)
            nc.gpsimd.dma_start(out=mb_t, in_=mb_d[:, :])
            wv_r = const.tile([128, DCH, 512], BF16, tag="wv")
            nc.gpsimd.dma_start(out=wv_r, in_=wv_d[:, :, :])
            wo_r = const.tile([128, PAIRS, DOUT], F32R, tag="wo")
            nc.gpsimd.dma_start(out=wo_r, in_=wo_d[:, :, :])
            bo_t = const.tile([1, DOUT], F32R)
            nc.gpsimd.dma_start(out=bo_t, in_=bo_d[:, :])
            ones_f = const.tile([128, 1], F32)
            nc.vector.memset(ones_f, 1.0)
            ones_r = const.tile([1, 128], F32R)
            nc.vector.tensor_copy(out=ones_r, in_=ones_f[0:1, 0:1].broadcast_to([1, 128]))
            bo_bc = const.tile([128, DOUT], F32)

            # streamed activations (sync DMA queue; xtk first: k/v proj needs it)
            xtk = xp.tile([128, DCH, KPAD], BF16, tag="xtk")
            nc.sync.dma_start(out=xtk, in_=xTk_d.rearrange("(c p) l -> p c l", p=128))
            xt = xp.tile([128, DCH, L], BF16, tag="xt")
            nc.sync.dma_start(out=xt, in_=xT_d.rearrange("(c p) l -> p c l", p=128))

            qT = big.tile([128, PAIRS, L], BF16, tag="qT")
            kT = big.tile([128, PAIRS, KPAD], BF16, tag="kT")
            vp = big.tile([128, KPC, HC, HD + 1], BF16, tag="vp")
            otn = big.tile([128, PAIRS, L], F32R, tag="otn")
            nc.vector.tensor_copy(
                out=vp[:, :, :, HD:HD + 1],
                in_=ones_f.unsqueeze(1).unsqueeze(1).broadcast_to([128, KPC, HC, 1]),
            )

            # ---- PE work-group emitters (each = one PSUM accumulation group) ----
            def kproj_group(mc, ci):
                c0, w = KST[ci], KCH[ci]
                ps = psF.tile([128, w], F32, tag="f", name="kproj_ps")
                for k in range(DCH):
                    nc.tensor.matmul(
                        ps, wqk_r[:, k, 512 + mc * 128:512 + (mc + 1) * 128],
                        xtk[:, k, c0:c0 + w], start=(k == 0), stop=(k == DCH - 1))
                nc.vector.tensor_scalar_add(
                    out=kT[:, mc, c0:c0 + w], in0=ps,
                    scalar1=bqk_t[:, PAIRS + mc:PAIRS + mc + 1])

            def vproj_group(kcg):
                ps = psF.tile([128, 512], F32, tag="f", name="vproj_ps")
                for k in range(DCH):
                    nc.tensor.matmul(
                        ps, xtk[:, k, kcg * 128:(kcg + 1) * 128],
                        wv_r[:, k, :], start=(k == 0), stop=(k == DCH - 1))
                nc.vector.tensor_copy(
                    out=vp[:, kcg, :, 0:HD],
                    in_=ps.rearrange("p (h d) -> p h d", h=HC))

            def qproj_group(mc, lc):
                ps = psF.tile([128, 512], F32, tag="f", name="qproj_ps")
                for k in range(DCH):
                    nc.tensor.matmul(
                        ps, wqk_r[:, k, mc * 128:(mc + 1) * 128],
                        xt[:, k, lc * 512:(lc + 1) * 512],
                        start=(k == 0), stop=(k == DCH - 1))
                nc.vector.tensor_scalar_add(
                    out=qT[:, mc, lc * 512:(lc + 1) * 512], in0=ps,
                    scalar1=bqk_t[:, mc:mc + 1])

            def bo_group(s):
                ps = psF.tile([128, 512], F32, tag="f", name="bo_ps")
                nc.tensor.matmul(ps, ones_r[0:1, :], bo_t[0:1, s * 512:(s + 1) * 512],
                                 start=True, stop=True)
                nc.vector.tensor_copy(out=bo_bc[:, s * 512:(s + 1) * 512], in_=ps)

            def outproj_qt(qt):
                y_sb = workS.tile([128, DOUT], F32, tag="y", name="y_sb")
                for dc in range(DC):
                    y_ps = psF.tile([128, 512], F32, tag="f", name="y_ps")
                    for i2 in range(PAIRS):
                        nc.tensor.matmul(
                            y_ps, otn[:, i2, qt * 128:(qt + 1) * 128],
                            wo_r[:, i2, dc * 512:(dc + 1) * 512],
                            start=(i2 == 0), stop=(i2 == PAIRS - 1))
                    nc.vector.tensor_tensor(
                        out=y_sb[:, dc * 512:(dc + 1) * 512],
                        in0=y_ps, in1=bo_bc[:, dc * 512:(dc + 1) * 512], op=ADD)
                nc.sync.dma_start(out=y_d[qt * 128:(qt + 1) * 128, :], in_=y_sb)

            # ---- pre-attention: k/v proj (all chunks), q proj pairs 0-1 ----
            for ci in range(len(KCH)):
                kproj_group(0, ci)
                kproj_group(1, ci)
                for sub in range(KCH[ci] // 128):
                    vproj_group(KST[ci] // 128 + sub)
            for lc in range(L // 512):
                qproj_group(0, lc)
                qproj_group(1, lc)
            for s in range(DC):
                bo_group(s)

            # filler: proj for pairs 2-3 pulled into attention gaps; out-proj
            # hemi-0 q-rows pulled into the qh=1 iterations.
            fillers = []
            for mc in (2, 3):
                for ci in range(len(KCH)):
                    fillers.append(lambda mc=mc, ci=ci: kproj_group(mc, ci))
                for lc in range(L // 512):
                    fillers.append(lambda mc=mc, lc=lc: qproj_group(mc, lc))
            fillers.reverse()      # pop() order = emission order

            def pull_fill(n):
                for _ in range(n):
                    if fillers:
                        fillers.pop()()

            # ---- attention ----
            outproj_done = 0
            prev_norm = [None]

            for it in range(QH * HC):
                qh, h = it // HC, it % HC
                i, j = h // 2, h % 2
                base = j * 64
                q0 = qh * 1024
                ot = psC.tile([HD + 1, 1024], F32, tag="ot", name="ot")
                prev = [None]

                def emit_pv(kc, pT):
                    for s in range(2):
                        nc.tensor.matmul(
                            ot[:, s * 512:(s + 1) * 512],
                            vp[:, kc, h, :], pT[:, s * 512:(s + 1) * 512],
                            start=(kc == 0), stop=(kc == KPC - 1))

                for kc in range(KPC):
                    st = psB.tile([128, 1024], F32, tag="st", name="st")
                    for s in range(2):
                        nc.tensor.matmul(
                            st[:, s * 512:(s + 1) * 512],
                            kT[base:base + 64, i, kc * 128:(kc + 1) * 128],
                            qT[base:base + 64, i, q0 + s * 512:q0 + (s + 1) * 512],
                            start=True, stop=True)
                    if kc == 1:
                        if prev_norm[0] is not None:
                            prev_norm[0]()
                            prev_norm[0] = None
                        # filler ahead of PV(0): covers the ot drain latency
                        if qh == 0 or not fillers:
                            pull_fill(2)
                        else:
                            pull_fill(1)
                        if qh == 1 and outproj_done < it - HC:
                            outproj_qt(outproj_done)
                            outproj_done += 1
                    if kc >= 1:
                        emit_pv(kc - 1, prev[0])
                    if kc == 5:
                        pull_fill(1)
                    pT = workP.tile([128, 1024], BF16, tag="pT", name="pT")
                    nc.scalar.activation(
                        out=pT, in_=st, func=AF.Exp,
                        bias=mb_t[:, kc:kc + 1], scale=1.0)
                    prev[0] = pT
                emit_pv(KPC - 1, prev[0])

                recip = workS.tile([1, 1024], F32R, tag="recip", name="recip")
                nc.vector.reciprocal(out=recip, in_=ot[HD:HD + 1, :])

                def make_norm(ot=ot, recip=recip, base=base, i=i, q0=q0):
                    def _norm():
                        bc_ps = psB.tile([64, 1024], F32, tag="st", name="bc_ps")
                        for s in range(2):
                            nc.tensor.matmul(
                                bc_ps[:, s * 512:(s + 1) * 512], ones_r[0:1, 0:64],
                                recip[0:1, s * 512:(s + 1) * 512], start=True, stop=True)
                        bc_sb = workS.tile([64, 1024], F32, tag="bc", name="bc_sb")
                        nc.vector.tensor_copy(out=bc_sb, in_=bc_ps)
                        nc.vector.tensor_tensor(
                            out=otn[base:base + 64, i, q0:q0 + 1024],
                            in0=ot[0:HD, :], in1=bc_sb, op=MULT)
                    return _norm

                prev_norm[0] = make_norm()

            prev_norm[0]()
            pull_fill(len(fillers))

            # ---- output projection (remaining q-rows) ----
            for qt in range(outproj_done, L // 128):
                outproj_qt(qt)

    # split multi-waits (walrus allows 1 sync wait per instruction reliably)
    if waitsplit:
        _split_excess_waits(nc)
    return nc


def _prep_inputs(x, mask, W_qkv, b_qkv, W_out, b_out, cfg):
    """Build the 8 per-core input maps (host-side shuffles)."""
    import ml_dtypes
    BF = ml_dtypes.bfloat16

    L, D, KPC = cfg["L"], cfg["D"], cfg["KPC"]
    HC, HD = cfg["HC"], cfg["HD"]
    DV = HC * HD              # 512 qkv dims per head-group
    KPAD = KPC * 128
    N = x.shape[0]
    scale = np.float32(1.0 / np.sqrt(HD))
    Wt = np.ascontiguousarray(W_qkv.T).astype(np.float32)    # [D, 3D]
    WoT = np.ascontiguousarray(W_out.T).astype(np.float32)   # [D, D]
    DCH = D // 128
    PAIRS = HC // 2

    # head-pair permutation: chunk mc, col c -> head 2mc + c//64, dim c%64
    idx = np.empty((PAIRS, 128), np.int64)
    for mc in range(PAIRS):
        c = np.arange(128)
        idx[mc] = (2 * mc + c // 64) * 64 + (c % 64)
    idxf = idx.reshape(-1)

    per_hg = []
    for hg in range(2):
        qs, ks, vs = hg * DV, D + hg * DV, 2 * D + hg * DV
        wq = Wt[:, qs:qs + DV][:, idxf] * scale
        wk = Wt[:, ks:ks + DV][:, idxf]
        wqk = np.concatenate([wq, wk], axis=1)                    # [D, 1024]
        wqk = np.ascontiguousarray(
            wqk.reshape(DCH, 128, 2 * DV).transpose(1, 0, 2)).astype(BF)
        wv = Wt[:, vs:vs + DV].reshape(DCH, 128, DV)
        wv = np.ascontiguousarray(wv.transpose(1, 0, 2)).astype(BF)
        bq = b_qkv[qs:qs + DV][idxf] * scale
        bk = b_qkv[ks:ks + DV][idxf]
        bqk = np.stack(
            [bq[mc * 128:(mc + 1) * 128] for mc in range(PAIRS)]
            + [bk[mc * 128:(mc + 1) * 128] for mc in range(PAIRS)], axis=1)
        bqk = np.ascontiguousarray(bqk).astype(np.float32)        # [128, 2*PAIRS]
        WoT_blk = WoT[hg * DV:(hg + 1) * DV, :]                   # [512, D]
        wo = np.ascontiguousarray(
            np.stack([WoT_blk[idx[i], :] for i in range(PAIRS)], axis=1))  # [128,4,D]
        # v-bias folds through softmax (weights sum to 1): bv @ WoT_blk
        bv = b_qkv[vs:vs + DV].astype(np.float32)
        bo_eff = bv @ WoT_blk
        if hg == 0:
            bo_eff = bo_eff + b_out.astype(np.float32)
        bo_eff = np.ascontiguousarray(bo_eff[None, :]).astype(np.float32)
        per_hg.append(dict(wqk=wqk, wv=wv, bqk=bqk, wo=wo, bo=bo_eff))

    xTs, xTks, mbs = [], [], []
    for n in range(N):
        xTs.append(np.ascontiguousarray(x[n].T).astype(BF))
        kept = np.nonzero(~mask[n])[0]
        xk = np.zeros((KPAD, D), np.float32)
        xk[:len(kept)] = x[n][kept]
        xTks.append(np.ascontiguousarray(xk.T).astype(BF))
        mb = np.full(KPAD, -1e9, np.float32)
        mb[:len(kept)] = 0.0
        mbs.append(np.ascontiguousarray(mb.reshape(KPC, 128).T))

    in_maps = []
    for c in range(2 * N):
        n, hg = c // 2, c % 2
        d = dict(per_hg[hg])
        d.update(xT=xTs[n], xTk=xTks[n], mb=mbs[n])
        in_maps.append(d)
    return in_maps


def kernel(x, mask, W_qkv, b_qkv, W_out, b_out):
    from concourse.bass_utils import run_bass_kernel_spmd

    x = np.asarray(x, dtype=np.float32)
    mask = np.asarray(mask).astype(bool)
    N, L, D = x.shape
    H = 16
    HD = D // H
    kept_max = int((~mask).sum(axis=1).max())
    KPC = max(2, -(-kept_max // 128))
    cfg = {"L": L, "D": D, "HC": H // 2, "HD": HD, "KPC": KPC}

    key = (L, D, H, KPC)
    if key not in _KERNEL_CACHE:
        _KERNEL_CACHE[key] = _build(cfg)
    nc = _KERNEL_CACHE[key]

    in_maps = _prep_inputs(
        x, mask,
        np.asarray(W_qkv, np.float32), np.asarray(b_qkv, np.float32),
        np.asarray(W_out, np.float32), np.asarray(b_out, np.float32), cfg,
    )
    res = run_bass_kernel_spmd(nc, in_maps, list(range(2 * N)))
    out = np.empty((N, L, D), np.float32)
    for n in range(N):
        out[n] = res.results[2 * n]["y"] + res.results[2 * n + 1]["y"]
    return out


# revision 9
# speedup vs baseline: 2.1165x; 1.0247x over previous
"""MiniMHSA Trainium2 kernel: 8 NeuronCores, shard = (batch n, head-group).

Reference computes, per batch n:
  qkv = x @ W_qkv.T + b_qkv ; split into q,k,v heads (H=16, HD=64)
  scores = (q @ k.T) / sqrt(HD), masked keys -> -1e9, softmax, @ v
  out = attn_out @ W_out.T + b_out

Core c handles n = c//2 and head-group hg = c%2 (8 heads each).

Key optimizations over the naive dataflow:
  * Mask compaction: masked keys contribute exactly 0 to softmax, so k/v are
    only computed for the ~50% unmasked keys (host gathers x columns, pads to
    a multiple of 128). Scores/PV/k-proj/v-proj all shrink proportionally.
  * Head-pair packing: two heads share the 128 partitions everywhere
    (partition p = (h%2)*64 + dim). Out-projection contracts 4 chunks of 128
    instead of 8 of 64.
  * v-bias + out-bias folded on host: softmax weights sum to 1, so the v bias
    contributes bv @ W_out.T — a constant folded into an effective out bias.
  * bf16 x/weights/attention operands (PE cost identical, halves SBUF/DMA);
    f32r out-projection.
  * No max-subtraction softmax: exp(s + mask_bias) directly (scores are O(5));
    denominator via an appended ones-column in v.
  * Pipelined attention: S(kc+1) ahead of PV(kc); normalization of iteration
    i-1 emitted inside iteration i; leftover projection / out-projection
    groups interleaved as PE filler during exp-paced attention.
"""
import sys

sys.path.insert(0, '/opt/trn_rl_repo')


import numpy as np

_KERNEL_CACHE = {}


def _split_excess_waits(nc):
    """Walrus codegen reliably accepts only ONE sync wait per instruction
    (Matmult hard-fails at 2, Drain at 5). Tile's scheduler can attach more.
    Move excess waits onto preceding same-engine NOPs — semantically identical
    since engine queues execute in order."""
    from concourse import mybir

    for f in nc.m.functions:
        for blk in f.blocks:
            il = blk.instructions
            i = 0
            while i < len(il):
                inst = il[i]
                si = inst.sync_info
                waits = list(si.on_wait) if si is not None and si.on_wait else []
                if len(waits) > 1:
                    keep = waits[-1:]
                    excess = waits[:-1]
                    pos = i
                    for j, wcond in enumerate(excess):
                        nop = mybir.InstNoOp(name=f"{inst.name}-ws{j}", ins=[], outs=[])
                        nop.engine = inst.engine
                        nop.sync_info = mybir.SyncInfo(on_wait=[wcond], on_update=[])
                        il.insert(pos, nop)
                        pos += 1
                        i += 1
                    inst.sync_info = mybir.SyncInfo(
                        on_wait=keep,
                        on_update=list(si.on_update) if si.on_update else [],
                    )
                i += 1


def _col_chunks(total):
    """Split a multiple-of-128 column count into chunks of 256..512 (each a
    multiple of 128) so fp32r matmuls stay >= 256 moving columns."""
    out, rem = [], total
    while rem > 512:
        step = 512 if rem - 512 >= 256 else 384
        out.append(step)
        rem -= step
    out.append(rem)
    return out


def _build(cfg, waitsplit=True):
    import concourse.bass as bass
    import concourse.tile as tile
    from concourse import mybir

    F32 = mybir.dt.float32
    F32R = mybir.dt.float32r
    BF16 = mybir.dt.bfloat16
    AF = mybir.ActivationFunctionType
    MULT = mybir.AluOpType.mult
    ADD = mybir.AluOpType.add

    L, D, KPC = cfg["L"], cfg["D"], cfg["KPC"]
    HC, HD = cfg["HC"], cfg["HD"]
    DCH = D // 128            # contraction chunks for projections
    PAIRS = HC // 2
    KPAD = KPC * 128
    QH = L // 1024            # attention q hemis (1024 wide)
    DOUT = D
    DC = DOUT // 512
    KCH = _col_chunks(KPAD)
    KST = [0]
    for w in KCH:
        KST.append(KST[-1] + w)

    nc = bass.Bass()
    xT_d = nc.dram_tensor("xT", [D, L], BF16, kind="ExternalInput")
    xTk_d = nc.dram_tensor("xTk", [D, KPAD], BF16, kind="ExternalInput")
    wqk_d = nc.dram_tensor("wqk", [128, DCH, 1024], BF16, kind="ExternalInput")
    wv_d = nc.dram_tensor("wv", [128, DCH, 512], BF16, kind="ExternalInput")
    bqk_d = nc.dram_tensor("bqk", [128, 2 * PAIRS], F32, kind="ExternalInput")
    mb_d = nc.dram_tensor("mb", [128, KPC], F32, kind="ExternalInput")
    wo_d = nc.dram_tensor("wo", [128, PAIRS, DOUT], F32, kind="ExternalInput")
    bo_d = nc.dram_tensor("bo", [1, DOUT], F32, kind="ExternalInput")
    y_d = nc.dram_tensor("y", [L, DOUT], F32, kind="ExternalOutput")

    with tile.TileContext(nc) as tc, \
         nc.allow_low_precision(reason="float32r/bf16 matmuls intended"):
        with tc.tile_pool(name="const", bufs=1) as const, \
             tc.tile_pool(name="big", bufs=1) as big, \
             tc.tile_pool(name="xp", bufs=1) as xp, \
             tc.tile_pool(name="workP", bufs=4) as workP, \
             tc.tile_pool(name="workS", bufs=2) as workS, \
             tc.tile_pool(name="psB", bufs=2, space="PSUM") as psB, \
             tc.tile_pool(name="psC", bufs=1, space="PSUM") as psC, \
             tc.tile_pool(name="psF", bufs=2, space="PSUM") as psF:

            # ---- constants / weights (pool DMA queue; k-weights first: the
            # first PE groups are k-proj and wait on them) ----
            wqk_r = const.tile([128, DCH, 1024], BF16, tag="wqk")
            nc.gpsimd.dma_start(out=wqk_r[:, :, 512:1024], in_=wqk_d[:, :, 512:1024])
            bqk_t = const.tile([128, 2 * PAIRS], F32)
            nc.gpsimd.dma_start(out=bqk_t, in_=bqk_d[:, :])
            mb_t = const.tile([128, KPC], F32)
            nc.gpsimd.dma_start(out=mb_t, in_=mb_d[:, :])
            wv_r = const.tile([128, DCH, 512], BF16, tag="wv")
            nc.gpsimd.dma_start(out=wv_r, in_=wv_d[:, :, :])
            nc.gpsimd.dma_start(out=wqk_r[:, :, 0:512], in_=wqk_d[:, :, 0:512])
            wo_r = const.tile([128, PAIRS, DOUT], F32R, tag="wo")
            nc.gpsimd.dma_start(out=wo_r, in_=wo_d[:, :, :])
            bo_t = const.tile([1, DOUT], F32R)
            nc.gpsimd.dma_start(out=bo_t, in_=bo_d[:, :])
            ones_f = const.tile([128, 1], F32)
            nc.vector.memset(ones_f, 1.0)
            ones_r = const.tile([1, 128], F32R)
            nc.vector.tensor_copy(out=ones_r, in_=ones_f[0:1, 0:1].broadcast_to([1, 128]))
            bo_bc = const.tile([128, DOUT], F32)

            # streamed activations (sync DMA queue; xtk per-chunk first so the
            # first k-proj group only waits ~2.4us)
            xtk = xp.tile([128, DCH, KPAD], BF16, tag="xtk")
            xt = xp.tile([128, DCH, L], BF16, tag="xt")
            xtk_re = xTk_d.rearrange("(c p) l -> p c l", p=128)
            for ci in range(len(KCH)):
                c0, w = KST[ci], KCH[ci]
                nc.sync.dma_start(out=xtk[:, :, c0:c0 + w], in_=xtk_re[:, :, c0:c0 + w])
            nc.sync.dma_start(out=xt, in_=xT_d.rearrange("(c p) l -> p c l", p=128))

            qT = big.tile([128, PAIRS, L], BF16, tag="qT")
            kT = big.tile([128, PAIRS, KPAD], BF16, tag="kT")
            vp = big.tile([128, KPC, HC, HD + 1], BF16, tag="vp")
            otn = big.tile([128, PAIRS, L], F32R, tag="otn")
            nc.vector.tensor_copy(
                out=vp[:, :, :, HD:HD + 1],
                in_=ones_f.unsqueeze(1).unsqueeze(1).broadcast_to([128, KPC, HC, 1]),
            )

            # ---- PE work-group emitters (each = one PSUM accumulation group) ----
            def kproj_group(mc, ci):
                c0, w = KST[ci], KCH[ci]
                ps = psF.tile([128, w], F32, tag="f", name="kproj_ps")
                for k in range(DCH):
                    nc.tensor.matmul(
                        ps, wqk_r[:, k, 512 + mc * 128:512 + (mc + 1) * 128],
                        xtk[:, k, c0:c0 + w], start=(k == 0), stop=(k == DCH - 1))
                nc.vector.tensor_scalar_add(
                    out=kT[:, mc, c0:c0 + w], in0=ps,
                    scalar1=bqk_t[:, PAIRS + mc:PAIRS + mc + 1])

            def vproj_group(kcg):
                ps = psF.tile([128, 512], F32, tag="f", name="vproj_ps")
                for k in range(DCH):
                    nc.tensor.matmul(
                        ps, xtk[:, k, kcg * 128:(kcg + 1) * 128],
                        wv_r[:, k, :], start=(k == 0), stop=(k == DCH - 1))
                nc.vector.tensor_copy(
                    out=vp[:, kcg, :, 0:HD],
                    in_=ps.rearrange("p (h d) -> p h d", h=HC))

            def qproj_group(mc, lc):
                ps = psF.tile([128, 512], F32, tag="f", name="qproj_ps")
                for k in range(DCH):
                    nc.tensor.matmul(
                        ps, wqk_r[:, k, mc * 128:(mc + 1) * 128],
                        xt[:, k, lc * 512:(lc + 1) * 512],
                        start=(k == 0), stop=(k == DCH - 1))
                nc.vector.tensor_scalar_add(
                    out=qT[:, mc, lc * 512:(lc + 1) * 512], in0=ps,
                    scalar1=bqk_t[:, mc:mc + 1])

            def bo_group(s):
                ps = psF.tile([128, 512], F32, tag="f", name="bo_ps")
                nc.tensor.matmul(ps, ones_r[0:1, :], bo_t[0:1, s * 512:(s + 1) * 512],
                                 start=True, stop=True)
                nc.vector.tensor_copy(out=bo_bc[:, s * 512:(s + 1) * 512], in_=ps)

            def outproj_qt(qt):
                y_sb = workS.tile([128, DOUT], F32, tag="y", name="y_sb")
                for dc in range(DC):
                    y_ps = psF.tile([128, 512], F32, tag="f", name="y_ps")
                    for i2 in range(PAIRS):
                        nc.tensor.matmul(
                            y_ps, otn[:, i2, qt * 128:(qt + 1) * 128],
                            wo_r[:, i2, dc * 512:(dc + 1) * 512],
                            start=(i2 == 0), stop=(i2 == PAIRS - 1))
                    nc.vector.tensor_tensor(
                        out=y_sb[:, dc * 512:(dc + 1) * 512],
                        in0=y_ps, in1=bo_bc[:, dc * 512:(dc + 1) * 512], op=ADD)
                nc.sync.dma_start(out=y_d[qt * 128:(qt + 1) * 128, :], in_=y_sb)

            # ---- pre-attention: k proj pairs 0-1, v proj (all), q proj pair 0 ----
            for ci in range(len(KCH)):
                kproj_group(0, ci)
                kproj_group(1, ci)
                for sub in range(KCH[ci] // 128):
                    vproj_group(KST[ci] // 128 + sub)
            for lc in range(L // 512):
                qproj_group(0, lc)
            for s in range(DC):
                bo_group(s)

            # filler: remaining proj pulled into attention gaps; out-proj
            # hemi-0 q-rows pulled into the qh=1 iterations. Order matters:
            # q pair 1 is needed from iteration 2, pair 2 from iteration 4...
            fillers = []
            for lc in range(L // 512):
                fillers.append(lambda lc=lc: qproj_group(1, lc))
            for mc in (2, 3):
                for ci in range(len(KCH)):
                    fillers.append(lambda mc=mc, ci=ci: kproj_group(mc, ci))
                for lc in range(L // 512):
                    fillers.append(lambda mc=mc, lc=lc: qproj_group(mc, lc))
            fillers.reverse()      # pop() order = emission order

            def pull_fill(n):
                for _ in range(n):
                    if fillers:
                        fillers.pop()()

            # ---- attention ----
            outproj_done = 0
            prev_norm = [None]

            for it in range(QH * HC):
                qh, h = it // HC, it % HC
                i, j = h // 2, h % 2
                base = j * 64
                q0 = qh * 1024
                ot = psC.tile([HD + 1, 1024], F32, tag="ot", name="ot")
                prev = [None]

                def emit_pv(kc, pT):
                    for s in range(2):
                        nc.tensor.matmul(
                            ot[:, s * 512:(s + 1) * 512],
                            vp[:, kc, h, :], pT[:, s * 512:(s + 1) * 512],
                            start=(kc == 0), stop=(kc == KPC - 1))

                for kc in range(KPC):
                    st = psB.tile([128, 1024], F32, tag="st", name="st")
                    for s in range(2):
                        nc.tensor.matmul(
                            st[:, s * 512:(s + 1) * 512],
                            kT[base:base + 64, i, kc * 128:(kc + 1) * 128],
                            qT[base:base + 64, i, q0 + s * 512:q0 + (s + 1) * 512],
                            start=True, stop=True)
                    if kc == 1:
                        if prev_norm[0] is not None:
                            prev_norm[0]()
                            prev_norm[0] = None
                        # filler ahead of PV(0): covers the ot drain latency
                        pull_fill(1)
                    if kc >= 1:
                        emit_pv(kc - 1, prev[0])
                    if kc in (3, 6):
                        pull_fill(1)
                        if kc == 3 and qh == 1 and not fillers \
                                and outproj_done < min(it - HC + 1, (L // 128) // 2):
                            outproj_qt(outproj_done)
                            outproj_done += 1
                    pT = workP.tile([128, 1024], BF16, tag="pT", name="pT")
                    nc.scalar.activation(
                        out=pT, in_=st, func=AF.Exp,
                        bias=mb_t[:, kc:kc + 1], scale=1.0)
                    prev[0] = pT
                emit_pv(KPC - 1, prev[0])

                recip = workS.tile([1, 1024], F32R, tag="recip", name="recip")
                nc.vector.reciprocal(out=recip, in_=ot[HD:HD + 1, :])

                def make_norm(ot=ot, recip=recip, base=base, i=i, q0=q0):
                    def _norm():
                        bc_ps = psB.tile([64, 1024], F32, tag="st", name="bc_ps")
                        for s in range(2):
                            nc.tensor.matmul(
                                bc_ps[:, s * 512:(s + 1) * 512], ones_r[0:1, 0:64],
                                recip[0:1, s * 512:(s + 1) * 512], start=True, stop=True)
                        bc_sb = workS.tile([64, 1024], F32, tag="bc", name="bc_sb")
                        nc.vector.tensor_copy(out=bc_sb, in_=bc_ps)
                        nc.vector.tensor_tensor(
                            out=otn[base:base + 64, i, q0:q0 + 1024],
                            in0=ot[0:HD, :], in1=bc_sb, op=MULT)
                    return _norm

                prev_norm[0] = make_norm()

            prev_norm[0]()
            pull_fill(len(fillers))

            # ---- output projection (remaining q-rows) ----
            for qt in range(outproj_done, L // 128):
                outproj_qt(qt)

    # split multi-waits (walrus allows 1 sync wait per instruction reliably)
    if waitsplit:
        _split_excess_waits(nc)
    return nc


def _prep_inputs(x, mask, W_qkv, b_qkv, W_out, b_out, cfg):
    """Build the 8 per-core input maps (host-side shuffles)."""
    import ml_dtypes
    BF = ml_dtypes.bfloat16

    L, D, KPC = cfg["L"], cfg["D"], cfg["KPC"]
    HC, HD = cfg["HC"], cfg["HD"]
    DV = HC * HD              # 512 qkv dims per head-group
    KPAD = KPC * 128
    N = x.shape[0]
    scale = np.float32(1.0 / np.sqrt(HD))
    Wt = np.ascontiguousarray(W_qkv.T).astype(np.float32)    # [D, 3D]
    WoT = np.ascontiguousarray(W_out.T).astype(np.float32)   # [D, D]
    DCH = D // 128
    PAIRS = HC // 2

    # head-pair permutation: chunk mc, col c -> head 2mc + c//64, dim c%64
    idx = np.empty((PAIRS, 128), np.int64)
    for mc in range(PAIRS):
        c = np.arange(128)
        idx[mc] = (2 * mc + c // 64) * 64 + (c % 64)
    idxf = idx.reshape(-1)

    per_hg = []
    for hg in range(2):
        qs, ks, vs = hg * DV, D + hg * DV, 2 * D + hg * DV
        wq = Wt[:, qs:qs + DV][:, idxf] * scale
        wk = Wt[:, ks:ks + DV][:, idxf]
        wqk = np.concatenate([wq, wk], axis=1)                    # [D, 1024]
        wqk = np.ascontiguousarray(
            wqk.reshape(DCH, 128, 2 * DV).transpose(1, 0, 2)).astype(BF)
        wv = Wt[:, vs:vs + DV].reshape(DCH, 128, DV)
        wv = np.ascontiguousarray(wv.transpose(1, 0, 2)).astype(BF)
        bq = b_qkv[qs:qs + DV][idxf] * scale
        bk = b_qkv[ks:ks + DV][idxf]
        bqk = np.stack(
            [bq[mc * 128:(mc + 1) * 128] for mc in range(PAIRS)]
            + [bk[mc * 128:(mc + 1) * 128] for mc in range(PAIRS)], axis=1)
        bqk = np.ascontiguousarray(bqk).astype(np.float32)        # [128, 2*PAIRS]
        WoT_blk = WoT[hg * DV:(hg + 1) * DV, :]                   # [512, D]
        wo = np.ascontiguousarray(
            np.stack([WoT_blk[idx[i], :] for i in range(PAIRS)], axis=1))  # [128,4,D]
        # v-bias folds through softmax (weights sum to 1): bv @ WoT_blk
        bv = b_qkv[vs:vs + DV].astype(np.float32)
        bo_eff = bv @ WoT_blk
        if hg == 0:
            bo_eff = bo_eff + b_out.astype(np.float32)
        bo_eff = np.ascontiguousarray(bo_eff[None, :]).astype(np.float32)
        per_hg.append(dict(wqk=wqk, wv=wv, bqk=bqk, wo=wo, bo=bo_eff))

    xTs, xTks, mbs = [], [], []
    for n in range(N):
        xTs.append(np.ascontiguousarray(x[n].T).astype(BF))
        kept = np.nonzero(~mask[n])[0]
        xk = np.zeros((KPAD, D), np.float32)
        xk[:len(kept)] = x[n][kept]
        xTks.append(np.ascontiguousarray(xk.T).astype(BF))
        mb = np.full(KPAD, -1e9, np.float32)
        mb[:len(kept)] = 0.0
        mbs.append(np.ascontiguousarray(mb.reshape(KPC, 128).T))

    in_maps = []
    for c in range(2 * N):
        n, hg = c // 2, c % 2
        d = dict(per_hg[hg])
        d.update(xT=xTs[n], xTk=xTks[n], mb=mbs[n])
        in_maps.append(d)
    return in_maps


def kernel(x, mask, W_qkv, b_qkv, W_out, b_out):
    from concourse.bass_utils import run_bass_kernel_spmd

    x = np.asarray(x, dtype=np.float32)
    mask = np.asarray(mask).astype(bool)
    N, L, D = x.shape
    H = 16
    HD = D // H
    kept_max = int((~mask).sum(axis=1).max())
    KPC = max(2, -(-kept_max // 128))
    cfg = {"L": L, "D": D, "HC": H // 2, "HD": HD, "KPC": KPC}

    key = (L, D, H, KPC)
    if key not in _KERNEL_CACHE:
        _KERNEL_CACHE[key] = _build(cfg)
    nc = _KERNEL_CACHE[key]

    in_maps = _prep_inputs(
        x, mask,
        np.asarray(W_qkv, np.float32), np.asarray(b_qkv, np.float32),
        np.asarray(W_out, np.float32), np.asarray(b_out, np.float32), cfg,
    )
    res = run_bass_kernel_spmd(nc, in_maps, list(range(2 * N)))
    out = np.empty((N, L, D), np.float32)
    for n in range(N):
        out[n] = res.results[2 * n]["y"] + res.results[2 * n + 1]["y"]
    return out


# revision 11
# speedup vs baseline: 2.2501x; 1.0631x over previous
"""MiniMHSA Trainium2 kernel: 8 NeuronCores, shard = (batch n, head-group).

Reference computes, per batch n:
  qkv = x @ W_qkv.T + b_qkv ; split into q,k,v heads (H=16, HD=64)
  scores = (q @ k.T) / sqrt(HD), masked keys -> -1e9, softmax, @ v
  out = attn_out @ W_out.T + b_out

Core c handles n = c//2 and head-group hg = c%2 (8 heads each).

Key optimizations over the naive dataflow:
  * Mask compaction: masked keys contribute exactly 0 to softmax, so k/v are
    only computed for the ~50% unmasked keys (host gathers x columns, pads to
    a multiple of 128). Scores/PV/k-proj/v-proj all shrink proportionally.
  * Head-pair packing: two heads share the 128 partitions everywhere
    (partition p = (h%2)*64 + dim). Out-projection contracts 4 chunks of 128
    instead of 8 of 64.
  * v-bias + out-bias folded on host: softmax weights sum to 1, so the v bias
    contributes bv @ W_out.T — a constant folded into an effective out bias.
  * bf16 x/weights/attention operands (PE cost identical, halves SBUF/DMA);
    f32r out-projection.
  * No max-subtraction softmax: exp(s + mask_bias) directly (scores are O(5));
    denominator via an appended ones-column in v.
  * Pipelined attention: S(kc+1) ahead of PV(kc); normalization of iteration
    i-1 emitted inside iteration i; leftover projection / out-projection
    groups interleaved as PE filler during exp-paced attention.
"""
import sys

sys.path.insert(0, '/opt/trn_rl_repo')


import numpy as np

_KERNEL_CACHE = {}


def _split_excess_waits(nc):
    """Walrus codegen reliably accepts only ONE sync wait per instruction
    (Matmult hard-fails at 2, Drain at 5). Tile's scheduler can attach more.
    Move excess waits onto preceding same-engine NOPs — semantically identical
    since engine queues execute in order."""
    from concourse import mybir

    for f in nc.m.functions:
        for blk in f.blocks:
            il = blk.instructions
            i = 0
            while i < len(il):
                inst = il[i]
                si = inst.sync_info
                waits = list(si.on_wait) if si is not None and si.on_wait else []
                if len(waits) > 1:
                    keep = waits[-1:]
                    excess = waits[:-1]
                    pos = i
                    for j, wcond in enumerate(excess):
                        nop = mybir.InstNoOp(name=f"{inst.name}-ws{j}", ins=[], outs=[])
                        nop.engine = inst.engine
                        nop.sync_info = mybir.SyncInfo(on_wait=[wcond], on_update=[])
                        il.insert(pos, nop)
                        pos += 1
                        i += 1
                    inst.sync_info = mybir.SyncInfo(
                        on_wait=keep,
                        on_update=list(si.on_update) if si.on_update else [],
                    )
                i += 1


def _col_chunks(total):
    """Split a multiple-of-128 column count into chunks of 256..512 (each a
    multiple of 128) so fp32r matmuls stay >= 256 moving columns."""
    out, rem = [], total
    while rem > 512:
        step = 512 if rem - 512 >= 256 else 384
        out.append(step)
        rem -= step
    out.append(rem)
    return out


def _build(cfg, waitsplit=True):
    import concourse.bass as bass
    import concourse.tile as tile
    from concourse import mybir

    F32 = mybir.dt.float32
    F32R = mybir.dt.float32r
    BF16 = mybir.dt.bfloat16
    AF = mybir.ActivationFunctionType
    MULT = mybir.AluOpType.mult
    ADD = mybir.AluOpType.add

    L, D, KPC = cfg["L"], cfg["D"], cfg["KPC"]
    HC, HD = cfg["HC"], cfg["HD"]
    DCH = D // 128            # contraction chunks for projections
    PAIRS = HC // 2
    KPAD = KPC * 128
    QH = L // 1024            # attention q hemis (1024 wide)
    DOUT = D
    DC = DOUT // 512
    KCH = _col_chunks(KPAD)
    KST = [0]
    for w in KCH:
        KST.append(KST[-1] + w)

    nc = bass.Bass()
    xT_d = nc.dram_tensor("xT", [D, L], BF16, kind="ExternalInput")
    xTk_d = nc.dram_tensor("xTk", [D, KPAD], BF16, kind="ExternalInput")
    wqk_d = nc.dram_tensor("wqk", [128, DCH, 1024], BF16, kind="ExternalInput")
    wv_d = nc.dram_tensor("wv", [128, DCH, 512], BF16, kind="ExternalInput")
    bqk_d = nc.dram_tensor("bqk", [128, 2 * PAIRS], F32, kind="ExternalInput")
    mb_d = nc.dram_tensor("mb", [128, KPC], F32, kind="ExternalInput")
    wo_d = nc.dram_tensor("wo", [128, PAIRS, DOUT], F32, kind="ExternalInput")
    bo_d = nc.dram_tensor("bo", [1, DOUT], F32, kind="ExternalInput")
    y_d = nc.dram_tensor("y", [L, DOUT], F32, kind="ExternalOutput")

    pbcast = cfg.get("PBCAST", True)
    with tile.TileContext(nc) as tc, \
         nc.allow_low_precision(reason="float32r/bf16 matmuls intended"):
        if pbcast:
            from concourse import library_config
            nc.gpsimd.load_library(library_config.attn)
        with tc.tile_pool(name="const", bufs=1) as const, \
             tc.tile_pool(name="big", bufs=1) as big, \
             tc.tile_pool(name="xp", bufs=1) as xp, \
             tc.tile_pool(name="workP", bufs=4) as workP, \
             tc.tile_pool(name="workS", bufs=2) as workS, \
             tc.tile_pool(name="psB", bufs=2, space="PSUM") as psB, \
             tc.tile_pool(name="psC", bufs=1, space="PSUM") as psC, \
             tc.tile_pool(name="psF", bufs=2, space="PSUM") as psF:

            # ---- constants / weights (pool DMA queue; k-weights first: the
            # first PE groups are k-proj and wait on them) ----
            wqk_r = const.tile([128, DCH, 1024], BF16, tag="wqk")
            nc.gpsimd.dma_start(out=wqk_r[:, :, 512:768], in_=wqk_d[:, :, 512:768])
            nc.gpsimd.dma_start(out=wqk_r[:, :, 768:1024], in_=wqk_d[:, :, 768:1024])
            bqk_t = const.tile([128, 2 * PAIRS], F32)
            nc.gpsimd.dma_start(out=bqk_t, in_=bqk_d[:, :])
            mb_t = const.tile([128, KPC], F32)
            nc.gpsimd.dma_start(out=mb_t, in_=mb_d[:, :])
            wv_r = const.tile([128, DCH, 512], BF16, tag="wv")
            nc.gpsimd.dma_start(out=wv_r, in_=wv_d[:, :, :])
            nc.gpsimd.dma_start(out=wqk_r[:, :, 0:512], in_=wqk_d[:, :, 0:512])
            wo_r = const.tile([128, PAIRS, DOUT], F32R, tag="wo")
            nc.gpsimd.dma_start(out=wo_r, in_=wo_d[:, :, :])
            bo_t = const.tile([1, DOUT], F32R)
            nc.gpsimd.dma_start(out=bo_t, in_=bo_d[:, :])
            ones_f = const.tile([128, 1], F32)
            nc.vector.memset(ones_f, 1.0)
            ones_r = const.tile([1, 128], F32R)
            nc.vector.tensor_copy(out=ones_r, in_=ones_f[0:1, 0:1].broadcast_to([1, 128]))
            bo_bc = const.tile([128, DOUT], F32)

            # streamed activations (sync DMA queue; xtk per-chunk first so the
            # first k-proj group only waits ~2.4us)
            xtk = xp.tile([128, DCH, KPAD], BF16, tag="xtk")
            xt = xp.tile([128, DCH, L], BF16, tag="xt")
            xtk_re = xTk_d.rearrange("(c p) l -> p c l", p=128)
            for ci in range(len(KCH)):
                c0, w = KST[ci], KCH[ci]
                nc.sync.dma_start(out=xtk[:, :, c0:c0 + w], in_=xtk_re[:, :, c0:c0 + w])
            nc.sync.dma_start(out=xt, in_=xT_d.rearrange("(c p) l -> p c l", p=128))

            qT = big.tile([128, PAIRS, L], BF16, tag="qT")
            kT = big.tile([128, PAIRS, KPAD], BF16, tag="kT")
            vp = big.tile([128, KPC, HC, HD + 1], BF16, tag="vp")
            otn = big.tile([128, PAIRS, L], F32R, tag="otn")
            nc.vector.tensor_copy(
                out=vp[:, :, :, HD:HD + 1],
                in_=ones_f.unsqueeze(1).unsqueeze(1).broadcast_to([128, KPC, HC, 1]),
            )

            # ---- PE work-group emitters (each = one PSUM accumulation group) ----
            def kproj_group(mc, ci):
                c0, w = KST[ci], KCH[ci]
                ps = psF.tile([128, w], F32, tag="f", name="kproj_ps")
                for k in range(DCH):
                    nc.tensor.matmul(
                        ps, wqk_r[:, k, 512 + mc * 128:512 + (mc + 1) * 128],
                        xtk[:, k, c0:c0 + w], start=(k == 0), stop=(k == DCH - 1))
                nc.vector.tensor_scalar_add(
                    out=kT[:, mc, c0:c0 + w], in0=ps,
                    scalar1=bqk_t[:, PAIRS + mc:PAIRS + mc + 1])

            def vproj_group(kcg):
                ps = psF.tile([128, 512], F32, tag="f", name="vproj_ps")
                for k in range(DCH):
                    nc.tensor.matmul(
                        ps, xtk[:, k, kcg * 128:(kcg + 1) * 128],
                        wv_r[:, k, :], start=(k == 0), stop=(k == DCH - 1))
                nc.vector.tensor_copy(
                    out=vp[:, kcg, :, 0:HD],
                    in_=ps.rearrange("p (h d) -> p h d", h=HC))

            def qproj_group(mc, lc):
                ps = psF.tile([128, 512], F32, tag="f", name="qproj_ps")
                for k in range(DCH):
                    nc.tensor.matmul(
                        ps, wqk_r[:, k, mc * 128:(mc + 1) * 128],
                        xt[:, k, lc * 512:(lc + 1) * 512],
                        start=(k == 0), stop=(k == DCH - 1))
                nc.vector.tensor_scalar_add(
                    out=qT[:, mc, lc * 512:(lc + 1) * 512], in0=ps,
                    scalar1=bqk_t[:, mc:mc + 1])

            def bo_group(s):
                ps = psF.tile([128, 512], F32, tag="f", name="bo_ps")
                nc.tensor.matmul(ps, ones_r[0:1, :], bo_t[0:1, s * 512:(s + 1) * 512],
                                 start=True, stop=True)
                nc.vector.tensor_copy(out=bo_bc[:, s * 512:(s + 1) * 512], in_=ps)

            def outproj_qt(qt):
                y_sb = workS.tile([128, DOUT], F32, tag="y", name="y_sb")
                for dc in range(DC):
                    y_ps = psF.tile([128, 512], F32, tag="f", name="y_ps")
                    for i2 in range(PAIRS):
                        nc.tensor.matmul(
                            y_ps, otn[:, i2, qt * 128:(qt + 1) * 128],
                            wo_r[:, i2, dc * 512:(dc + 1) * 512],
                            start=(i2 == 0), stop=(i2 == PAIRS - 1))
                    nc.vector.tensor_tensor(
                        out=y_sb[:, dc * 512:(dc + 1) * 512],
                        in0=y_ps, in1=bo_bc[:, dc * 512:(dc + 1) * 512], op=ADD)
                nc.sync.dma_start(out=y_d[qt * 128:(qt + 1) * 128, :], in_=y_sb)

            # ---- pre-attention: k proj pairs 0-1, v proj (all), q proj pair 0 ----
            for ci in range(len(KCH)):
                kproj_group(0, ci)
                kproj_group(1, ci)
                for sub in range(KCH[ci] // 128):
                    vproj_group(KST[ci] // 128 + sub)
            for lc in range(L // 512):
                qproj_group(0, lc)
            for s in range(DC):
                bo_group(s)

            # filler: remaining proj pulled into attention gaps; out-proj
            # hemi-0 q-rows pulled into the qh=1 iterations. Order matters:
            # q pair 1 is needed from iteration 2, pair 2 from iteration 4...
            fillers = []
            for lc in range(L // 512):
                fillers.append(lambda lc=lc: qproj_group(1, lc))
            for mc in (2, 3):
                for ci in range(len(KCH)):
                    fillers.append(lambda mc=mc, ci=ci: kproj_group(mc, ci))
                for lc in range(L // 512):
                    fillers.append(lambda mc=mc, lc=lc: qproj_group(mc, lc))
            fillers.reverse()      # pop() order = emission order

            def pull_fill(n):
                for _ in range(n):
                    if fillers:
                        fillers.pop()()

            # ---- attention ----
            # qh0 filler rationing: front-loaded (proj deadlines), exact supply
            QH0_PULLS = {0: (1, 3, 6), 1: (1, 3, 6), 2: (1, 3, 6), 3: (1, 3, 6),
                         4: (1, 3), 5: (1, 3), 6: (1,), 7: (1,)}
            outproj_done = 0
            prev_norm = [None]

            for it in range(QH * HC):
                qh, h = it // HC, it % HC
                i, j = h // 2, h % 2
                base = j * 64
                q0 = qh * 1024
                ot = psC.tile([HD + 1, 1024], F32, tag="ot", name="ot")
                prev = [None]

                def emit_pv(kc, pT):
                    for s in range(2):
                        nc.tensor.matmul(
                            ot[:, s * 512:(s + 1) * 512],
                            vp[:, kc, h, :], pT[:, s * 512:(s + 1) * 512],
                            start=(kc == 0), stop=(kc == KPC - 1))

                def pull_slot(kc):
                    if qh == 0:
                        if kc in QH0_PULLS.get(it, (1,)):
                            pull_fill(1)
                        return
                    if fillers and kc in (1, 3):
                        pull_fill(1)
                        return
                    if kc in (1, 3) and outproj_done < min(it - HC + 1, (L // 128) // 2):
                        outproj_qt(outproj_done)
                        _op_inc()

                def _op_inc():
                    nonlocal outproj_done
                    outproj_done += 1

                for kc in range(KPC):
                    st = psB.tile([128, 1024], F32, tag="st", name="st")
                    for s in range(2):
                        nc.tensor.matmul(
                            st[:, s * 512:(s + 1) * 512],
                            kT[base:base + 64, i, kc * 128:(kc + 1) * 128],
                            qT[base:base + 64, i, q0 + s * 512:q0 + (s + 1) * 512],
                            start=True, stop=True)
                    if kc == 1 and prev_norm[0] is not None:
                        prev_norm[0]()
                        prev_norm[0] = None
                    if kc in (1, 3, 6):
                        pull_slot(kc)
                    if kc >= 1:
                        emit_pv(kc - 1, prev[0])
                    pT = workP.tile([128, 1024], BF16, tag="pT", name="pT")
                    nc.scalar.activation(
                        out=pT, in_=st, func=AF.Exp,
                        bias=mb_t[:, kc:kc + 1], scale=1.0)
                    prev[0] = pT
                emit_pv(KPC - 1, prev[0])

                recip = workS.tile([1, 1024], F32R, tag="recip", name="recip")
                nc.vector.reciprocal(out=recip, in_=ot[HD:HD + 1, :])

                if pbcast:
                    # normalization entirely off the PE: Pool broadcast + DVE
                    bc_sb = workS.tile([64, 1024], F32R, tag="bc", name="bc_sb")
                    nc.gpsimd.partition_broadcast(out_ap=bc_sb, in_ap=recip)
                    nc.vector.tensor_tensor(
                        out=otn[base:base + 64, i, q0:q0 + 1024],
                        in0=ot[0:HD, :], in1=bc_sb, op=MULT)
                else:
                    def make_norm(ot=ot, recip=recip, base=base, i=i, q0=q0):
                        def _norm():
                            bc_ps = psB.tile([64, 1024], F32, tag="st", name="bc_ps")
                            for s in range(2):
                                nc.tensor.matmul(
                                    bc_ps[:, s * 512:(s + 1) * 512], ones_r[0:1, 0:64],
                                    recip[0:1, s * 512:(s + 1) * 512],
                                    start=True, stop=True)
                            bc_sb = workS.tile([64, 1024], F32, tag="bc", name="bc_sb")
                            nc.vector.tensor_copy(out=bc_sb, in_=bc_ps)
                            nc.vector.tensor_tensor(
                                out=otn[base:base + 64, i, q0:q0 + 1024],
                                in0=ot[0:HD, :], in1=bc_sb, op=MULT)
                        return _norm

                    prev_norm[0] = make_norm()

            if prev_norm[0] is not None:
                prev_norm[0]()
            pull_fill(len(fillers))

            # ---- output projection (remaining q-rows) ----
            for qt in range(outproj_done, L // 128):
                outproj_qt(qt)

    # split multi-waits (walrus allows 1 sync wait per instruction reliably)
    if waitsplit:
        _split_excess_waits(nc)
    return nc


def _prep_inputs(x, mask, W_qkv, b_qkv, W_out, b_out, cfg):
    """Build the 8 per-core input maps (host-side shuffles)."""
    import ml_dtypes
    BF = ml_dtypes.bfloat16

    L, D, KPC = cfg["L"], cfg["D"], cfg["KPC"]
    HC, HD = cfg["HC"], cfg["HD"]
    DV = HC * HD              # 512 qkv dims per head-group
    KPAD = KPC * 128
    N = x.shape[0]
    scale = np.float32(1.0 / np.sqrt(HD))
    Wt = np.ascontiguousarray(W_qkv.T).astype(np.float32)    # [D, 3D]
    WoT = np.ascontiguousarray(W_out.T).astype(np.float32)   # [D, D]
    DCH = D // 128
    PAIRS = HC // 2

    # head-pair permutation: chunk mc, col c -> head 2mc + c//64, dim c%64
    idx = np.empty((PAIRS, 128), np.int64)
    for mc in range(PAIRS):
        c = np.arange(128)
        idx[mc] = (2 * mc + c // 64) * 64 + (c % 64)
    idxf = idx.reshape(-1)

    per_hg = []
    for hg in range(2):
        qs, ks, vs = hg * DV, D + hg * DV, 2 * D + hg * DV
        wq = Wt[:, qs:qs + DV][:, idxf] * scale
        wk = Wt[:, ks:ks + DV][:, idxf]
        wqk = np.concatenate([wq, wk], axis=1)                    # [D, 1024]
        wqk = np.ascontiguousarray(
            wqk.reshape(DCH, 128, 2 * DV).transpose(1, 0, 2)).astype(BF)
        wv = Wt[:, vs:vs + DV].reshape(DCH, 128, DV)
        wv = np.ascontiguousarray(wv.transpose(1, 0, 2)).astype(BF)
        bq = b_qkv[qs:qs + DV][idxf] * scale
        bk = b_qkv[ks:ks + DV][idxf]
        bqk = np.stack(
            [bq[mc * 128:(mc + 1) * 128] for mc in range(PAIRS)]
            + [bk[mc * 128:(mc + 1) * 128] for mc in range(PAIRS)], axis=1)
        bqk = np.ascontiguousarray(bqk).astype(np.float32)        # [128, 2*PAIRS]
        WoT_blk = WoT[hg * DV:(hg + 1) * DV, :]                   # [512, D]
        wo = np.ascontiguousarray(
            np.stack([WoT_blk[idx[i], :] for i in range(PAIRS)], axis=1))  # [128,4,D]
        # v-bias folds through softmax (weights sum to 1): bv @ WoT_blk
        bv = b_qkv[vs:vs + DV].astype(np.float32)
        bo_eff = bv @ WoT_blk
        if hg == 0:
            bo_eff = bo_eff + b_out.astype(np.float32)
        bo_eff = np.ascontiguousarray(bo_eff[None, :]).astype(np.float32)
        per_hg.append(dict(wqk=wqk, wv=wv, bqk=bqk, wo=wo, bo=bo_eff))

    xTs, xTks, mbs = [], [], []
    for n in range(N):
        xTs.append(np.ascontiguousarray(x[n].T).astype(BF))
        kept = np.nonzero(~mask[n])[0]
        xk = np.zeros((KPAD, D), np.float32)
        xk[:len(kept)] = x[n][kept]
        xTks.append(np.ascontiguousarray(xk.T).astype(BF))
        mb = np.full(KPAD, -1e9, np.float32)
        mb[:len(kept)] = 0.0
        mbs.append(np.ascontiguousarray(mb.reshape(KPC, 128).T))

    in_maps = []
    for c in range(2 * N):
        n, hg = c // 2, c % 2
        d = dict(per_hg[hg])
        d.update(xT=xTs[n], xTk=xTks[n], mb=mbs[n])
        in_maps.append(d)
    return in_maps


def kernel(x, mask, W_qkv, b_qkv, W_out, b_out):
    from concourse.bass_utils import run_bass_kernel_spmd

    x = np.asarray(x, dtype=np.float32)
    mask = np.asarray(mask).astype(bool)
    N, L, D = x.shape
    H = 16
    HD = D // H
    kept_max = int((~mask).sum(axis=1).max())
    KPC = max(2, -(-kept_max // 128))
    cfg = {"L": L, "D": D, "HC": H // 2, "HD": HD, "KPC": KPC}

    key = (L, D, H, KPC)
    if key not in _KERNEL_CACHE:
        _KERNEL_CACHE[key] = _build(cfg)
    nc = _KERNEL_CACHE[key]

    in_maps = _prep_inputs(
        x, mask,
        np.asarray(W_qkv, np.float32), np.asarray(b_qkv, np.float32),
        np.asarray(W_out, np.float32), np.asarray(b_out, np.float32), cfg,
    )
    res = run_bass_kernel_spmd(nc, in_maps, list(range(2 * N)))
    out = np.empty((N, L, D), np.float32)
    for n in range(N):
        out[n] = res.results[2 * n]["y"] + res.results[2 * n + 1]["y"]
    return out
